# revision 25
# baseline (speedup 1.0000x reference)
"""Causal self-attention (B=4, T=2048, C=1024, 16 heads) on 8 TRN2 NeuronCores.

Sharding: data-parallel over batch (4) x tensor-parallel over heads (2 groups
of 8). Each core computes qkv + attention for its 8 heads and a partial
output projection (row-parallel); the host sums the two partials per batch.

Everything on-chip runs in a transposed layout so no tensor ever needs an
on-device transpose:
  QT/KT [ch, t]  <- W^T @ x^T      (x^T prepared on host)
  attT  [k, q]   = exp(K @ Q^T / 8) * causal_mask
  yT    [ch, q]  = V_aug^T @ attT  (V augmented with a ones column per head ->
                                    row 64 of each head's block = softmax denom)
  out   [q, c]   = yT^T @ Wp       (partial; host-reduced across head groups)

Softmax skips max-subtraction (scores/8 are O(1) here, exp is safe), which is
mathematically identical to the reference; fully-masked blocks are never
computed, straddle blocks only compute the causally valid column range, and
only the diagonal 128-wide sub-block needs a mask multiply (on GpSimd, which
is otherwise idle).

All matmul operands are fp16 (same 1 cyc/row PE rate as fp32r at N>=256 but
no 4x penalty at N=128, and half the DMA/SBUF traffic); PSUM accumulation and
the softmax normalization stay f32. Phase B is software-pipelined per k-tile:
QK+exp issue ahead, AV lags LAG tiles so the in-order PE queue never waits on
the Scalar-engine exp; the normalization units of each head-pair group are
displaced into the next group's tile stream for the same reason.
"""

import os
import sys

import numpy as np

for _p in ("/opt/trn_rl_repo", "/root/.axon_site/_ro/trn_rl_repo"):
    if os.path.isdir(_p) and _p not in sys.path:
        sys.path.append(_p)

import concourse.bass as bass  # noqa: E402,F401
import concourse.mybir as mybir  # noqa: E402
import concourse.tile as tile  # noqa: E402
from concourse import bacc, bass_utils  # noqa: E402

f32 = mybir.dt.float32
f32r = mybir.dt.float32r
F16 = mybir.dt.float16

B, T, C = 4, 2048, 1024
N_HEAD, D = 16, 64
NCORES = 8
HPC = 8  # heads per core
CH = HPC * D  # 512 channels per core
P = 128
NQ = 512  # q-strip width
NSTRIP = T // NQ  # 4
SCALE = 1.0 / 8.0  # 1/sqrt(D)
LAG = 5  # AV trails QK/exp by this many k-tiles in the PE stream

MM_DT = F16

# exp(u/8) ~= p(u)^8 with p = rel-minimax quadratic of exp(u/64) on
# [-21.5, 21.5] (measured max |raw score| is 20.37). Runs on the DVE as a
# custom op (8-stage budget: quad + 3 squarings) so the Scalar engine's exp
# stream stops being the critical path in the late (long-context) strips.
# Max rel err 1.3e-2 on ~1/4 of the attention weights -> ~2e-3 on the output.
EK0 = 1.000390887260437
EK1 = 0.01584288291633129
EK2 = 0.0001212068455060944


def register_exp_op():
    """Register the EXP8_ANT custom DVE op (idempotent)."""
    from concourse import dve_ops as D
    from concourse.dve_spec import Spec, Src0, sq, C0, C1, C2, _has_src1, lower
    from concourse.dve_uop import DveOpSpec

    for op in D.OPS:
        if op.name == "EXP8_ANT":
            return op

    _p = (C0 + Src0 * C1) + sq(Src0) * C2
    _body = sq(sq(sq(_p)))

    def _ref(in0, in1, s0, s1, imm2):
        p = (s0 + in0 * s1) + in0 * in0 * imm2
        return ((p * p) ** 2) ** 2

    spec = Spec(body=_body, reference=_ref)
    op = D.DveOp("EXP8_ANT", spec, subdim=False, uops_sha={})
    D.OPS.append(op)
    # import-time derived registries must be refreshed for the new row
    D._SUB_OPCODE_FOR_NAME[op.name] = D._CUSTOM_DVE_ROW_BASE + len(D.OPS) - 1
    D.CUSTOM_DVE_SPECS[op.name] = spec
    # pre-seed the compile cache so the empty sha dict is never checked
    for ver in ("v3", "v4"):
        try:
            uops = lower(spec, ver=ver)
        except Exception:
            continue
        D._COMPILE_CACHE[(op.name, ver)] = DveOpSpec(
            name=op.name,
            opcode=D.get_dve_sub_opcode(op.name),
            uops=uops,
            rd1_en=_has_src1(spec),
        )
    return op


def build():
    nc = bacc.Bacc("TRN2", target_bir_lowering=False, debug=False)
    # All operands arrive pre-packed host-side as [128, n*512] with the
    # 128-partition c-blocks stacked along columns: every DMA then moves
    # 8KB/partition of contiguous dram per tile instead of 1KB rows, which
    # cuts descriptor count 8x and gets the first QT chain its inputs
    # ~6us earlier.
    xt = nc.dram_tensor("xt", (P, NSTRIP * 8 * NQ), MM_DT, kind="ExternalInput")
    wq = nc.dram_tensor("wq", (P, 8 * CH), MM_DT, kind="ExternalInput")
    wk = nc.dram_tensor("wk", (P, 8 * CH), MM_DT, kind="ExternalInput")
    wv = nc.dram_tensor("wv", (P, 8 * CH), MM_DT, kind="ExternalInput")
    wp = nc.dram_tensor("wp", (P, 8 * NQ), MM_DT, kind="ExternalInput")
    # mk: cols 0-127 identity, cols 128-255 additive causal mask
    # (0 keep / -60000 drop)
    mk = nc.dram_tensor("mk", (P, 256), F16, kind="ExternalInput")
    # out packed the same way: strip s holds blocks k=o*2+n (o = q-tile
    # within strip, n = c-col half) at cols (s*8+k)*NQ
    out = nc.dram_tensor("out", (P, 32 * NQ), F16, kind="ExternalOutput")
    Exp = mybir.ActivationFunctionType.Exp
    exp_op = register_exp_op()

    with tile.TileContext(nc) as tc:
        with (
            tc.tile_pool(name="sb", bufs=1) as sb,
            tc.tile_pool(name="ps", bufs=1, space="PSUM") as psp,
        ):
            mask = sb.tile([P, 256], F16, tag="mask", bufs=1, name="mask")
            nc.sync.dma_start(mask[:], mk[:])
            maskmul = mask[:, 0:128]
            # PE warmup: stream zeros through the PE while the weight/x DMAs
            # land so the tensor engine p-state ramps before real work.
            # memset on GpSimd (its preamble finishes earliest and nothing
            # else queues there) so the first warmup matmul isn't gated on
            # the DVE preamble + memsets. ~14 cold matmuls (~6us) cover the
            # HAM busy-window flip; the DMAs land at about the same time.
            wrm = sb.tile([P, NQ], F16, tag="wrm", bufs=1, name="wrm")
            nc.gpsimd.memset(wrm[:], 0.0)
            for i in range(28):
                wps_ = psp.tile([P, NQ], f32, tag="mm", bufs=2, name="wrmp")
                nc.tensor.matmul(wps_[:], wrm[:, 0:P], wrm[:], start=True, stop=True)
            sel = sb.tile([33, P], F16, tag="sel", bufs=1, name="sel")
            nc.vector.memset(sel[:], 0.0)
            nc.vector.memset(sel[0:1, 0:64], 1.0)
            nc.vector.memset(sel[32:33, 64:128], 1.0)
            col1 = sb.tile([P, HPC], F16, tag="ones8", bufs=1, name="col1")
            nc.vector.memset(col1[:], 1.0)
            # den rows 1..31 must be a harmless non-zero constant for the
            # reciprocal (sel zeroes them in the broadcast); memset once,
            # rows 0/32 are overwritten per group.
            dens = [
                sb.tile([33, NQ], f32, tag="den", bufs=2, name=f"den{i}")
                for i in range(2)
            ]
            for dt_ in dens:
                nc.vector.memset(dt_[:], 1.0)

            # ---- weight + first-strip x loads, interleaved so the first QT
            # accumulation chain unblocks after two small DMAs. x issues on
            # the (startup-idle) Activation engine in parallel with Sync.
            wq_sb, wk_sb, wv_sb = ([] for _ in range(3))
            xts = {}  # strip -> list of 8 c-tiles
            xdma_done = set()

            def x_dma(s, eng):
                if s in xdma_done or s >= NSTRIP:
                    return
                xdma_done.add(s)
                t = sb.tile([P, 8 * NQ], MM_DT, tag="x", bufs=3, name=f"x{s}")
                eng.dma_start(t[:], xt[:, s * 8 * NQ : (s + 1) * 8 * NQ])
                xts[s] = [t[:, c * NQ : (c + 1) * NQ] for c in range(8)]

            wqbig = sb.tile([P, 8 * CH], MM_DT, tag="w", bufs=4, name="wq")
            nc.sync.dma_start(wqbig[:], wq[:])
            wq_sb = [wqbig[:, c * CH : (c + 1) * CH] for c in range(8)]
            x_dma(0, nc.scalar)
            wkbig = sb.tile([P, 8 * CH], MM_DT, tag="w", bufs=4, name="wk")
            nc.sync.dma_start(wkbig[:], wk[:])
            wk_sb = [wkbig[:, c * CH : (c + 1) * CH] for c in range(8)]
            wvbig = sb.tile([P, 8 * CH], MM_DT, tag="w", bufs=4, name="wv")
            nc.sync.dma_start(wvbig[:], wv[:])
            wv_sb = [wvbig[:, c * CH : (c + 1) * CH] for c in range(8)]
            wpbig = sb.tile([P, 8 * NQ], MM_DT, tag="w", bufs=4, name="wp")
            nc.scalar.dma_start(wpbig[:], wp[:])
            wp_sb = [wpbig[:, k * NQ : (k + 1) * NQ] for k in range(8)]

            kts = [sb.tile([P, T], MM_DT, tag="kt", bufs=4, name=f"kt{m}") for m in range(4)]
            vts = [None] * 16
            qts = {}

            # ---- Phase A: QT (strip-local), KT (transposed) and V (ones-augmented)
            def a_units(s):
                def u_dma():
                    x_dma(s, nc.sync)
                    x_dma(s + 1, nc.sync)
                    qts[s] = []

                def u_qt(m):
                    ps = psp.tile([P, NQ], f32, tag="mm", bufs=2, name="psa")
                    for c in range(8):
                        nc.tensor.matmul(
                            ps[:],
                            wq_sb[c][:, m * P : (m + 1) * P],
                            xts[s][c][:],
                            start=(c == 0),
                            stop=(c == 7),
                        )
                    qt_t = sb.tile([P, NQ], MM_DT, tag="qt", bufs=8, name=f"q{s}_{m}")
                    nc.vector.tensor_copy(qt_t[:], ps[:])
                    qts[s].append(qt_t)

                def u_kt(m):
                    ps = psp.tile([P, NQ], f32, tag="mm", bufs=2, name="psk")
                    for c in range(8):
                        nc.tensor.matmul(
                            ps[:],
                            wk_sb[c][:, m * P : (m + 1) * P],
                            xts[s][c][:],
                            start=(c == 0),
                            stop=(c == 7),
                        )
                    nc.vector.tensor_copy(kts[m][:, s * NQ : (s + 1) * NQ], ps[:])

                def u_v(mt):
                    g = s * 4 + mt
                    ps = psp.tile([P, NQ], f32, tag="mm", bufs=2, name="psv")
                    for c in range(8):
                        nc.tensor.matmul(
                            ps[:],
                            xts[s][c][:, mt * P : (mt + 1) * P],
                            wv_sb[c][:],
                            start=(c == 0),
                            stop=(c == 7),
                        )
                    vt = sb.tile([P, HPC * 65], F16, tag="v", bufs=16, name=f"v{g}")
                    v3 = vt.rearrange("p (h e) -> p h e", e=65)
                    # V copy must stay off ACT: queued ahead of ready exps it
                    # head-of-line blocks the attention pipeline.
                    nc.vector.tensor_copy(v3[:, :, 0:64], ps.rearrange("p (h e) -> p h e", e=64))
                    nc.vector.tensor_copy(
                        v3[:, :, 64:65], col1[:].rearrange("p (h e) -> p h e", e=1)
                    )
                    vts[g] = vt

                units = [u_dma]
                for m in range(4):
                    units.append(lambda m=m: u_qt(m))
                    units.append(lambda m=m: u_kt(m))
                    units.append(lambda m=m: u_v(m))
                return units

            # ---- Phase B: flash attention, software-pipelined per k-tile.
            yts = {}
            pending_norms = []  # norm units of the previous head-pair group

            def bc_units(s, c):
                nkt = 4 * (s + 1)
                st = {}

                def u_tile(kt):
                    j = kt - 4 * s  # >=0: diagonal-straddling tile
                    off = 128 * max(j, 0)
                    qkp = psp.tile([P, 2 * NQ], f32, tag="qk", bufs=2, name="qkp")
                    att = sb.tile([P, 2 * NQ], F16, tag="att", bufs=LAG + 3, name="att")
                    for sub in range(2):
                        nc.tensor.matmul(
                            qkp[:, sub * NQ + off : (sub + 1) * NQ],
                            kts[c][sub * 64 : (sub + 1) * 64, kt * P : (kt + 1) * P],
                            qts[s][c][sub * 64 : (sub + 1) * 64, off:NQ],
                            start=True,
                            stop=True,
                        )
                    # late strips are Scalar-exp-bound: route ~1/4 of the
                    # k-tiles' exps through the DVE poly op instead.
                    dve_exp = False
                    if dve_exp:
                        nc.vector._custom_dve(
                            exp_op,
                            out=att.rearrange("p (u q) -> p u q", u=2)[:, :, off:NQ],
                            in0=qkp.rearrange("p (u q) -> p u q", u=2)[:, :, off:NQ],
                            s0=EK0,
                            s1=EK1,
                            imm2=EK2,
                        )
                    else:
                        nc.scalar.activation(
                            att.rearrange("p (u q) -> p u q", u=2)[:, :, off:NQ],
                            qkp.rearrange("p (u q) -> p u q", u=2)[:, :, off:NQ],
                            Exp,
                            scale=SCALE,
                        )
                    if j >= 0:
                        for sub in range(2):
                            nc.gpsimd.tensor_mul(
                                att[:, sub * NQ + off : sub * NQ + off + 128],
                                att[:, sub * NQ + off : sub * NQ + off + 128],
                                maskmul,
                            )
                    st[kt] = (att, off)

                def u_av(kt):
                    if "av" not in st:
                        st["av"] = [
                            psp.tile([65, NQ], f32, tag="sm", bufs=2, name=f"av{s}{c}{u}")
                            for u in range(2)
                        ]
                    att, off = st.pop(kt)
                    for sub in range(2):
                        h = 2 * c + sub
                        nc.tensor.matmul(
                            st["av"][sub][:, off:NQ],
                            vts[kt][:, h * 65 : (h + 1) * 65],
                            att[:, sub * NQ + off : (sub + 1) * NQ],
                            start=(kt == 0),
                            stop=(kt == nkt - 1),
                        )

                def u_norm_a():
                    # denominators -> reciprocals (DVE), cast f16 for the
                    # 1 cyc/row selector matmul
                    av = st["av"]
                    den = dens[(4 * s + c) % 2]
                    rcf = sb.tile([33, NQ], f32, tag="rcf", bufs=2, name="rcf")
                    rc2 = sb.tile([33, NQ], F16, tag="rc2", bufs=2, name="rc2")
                    for sub in range(2):
                        nc.vector.tensor_copy(
                            den[32 * sub : 32 * sub + 1, :], av[sub][64:65, :]
                        )
                    nc.vector.reciprocal_approx_fast(out=rcf[:], in_=den[:])
                    nc.vector.tensor_copy(rc2[:], rcf[:])
                    st["rc2"] = rc2

                def u_norm_b():
                    # broadcast reciprocals to 128 partitions via selector matmul
                    bc_ps = psp.tile([P, NQ], f32, tag="mm", bufs=2, name="bcp")
                    nc.tensor.matmul(bc_ps[:], sel[:], st["rc2"][:], start=True, stop=True)
                    st["bc_ps"] = bc_ps

                def u_norm_c():
                    bc = sb.tile([P, NQ], f32, tag="bc", bufs=2, name="bc")
                    nc.vector.tensor_copy(bc[:], st["bc_ps"][:])
                    av = st["av"]
                    yts[(c, s)] = sb.tile([P, NQ], MM_DT, tag="yt", bufs=16, name=f"y{c}{s}")
                    for sub in range(2):
                        nc.vector.tensor_mul(
                            yts[(c, s)][sub * 64 : (sub + 1) * 64, :],
                            av[sub][0:64, :],
                            bc[sub * 64 : (sub + 1) * 64, :],
                        )

                units = []
                for kt in range(nkt):
                    def u(kt=kt):
                        u_tile(kt)
                        if kt - LAG >= 0:
                            u_av(kt - LAG)
                    units.append(u)
                # AV tail (exp-gated) + norm chain are displaced into the
                # next group's tile stream: its early QKs are independent of
                # this group's last exps, so the PE never idles on them.
                # Every tail unit carries a PE instruction (norm_a rides with
                # the last AV, norm_c with the bcast): a PE-less unit between
                # two QK pairs makes the pairs adjacent in the PE queue, and
                # back-to-back 64-row-mode pairs stall ~450ns on the drain.
                tail = [lambda kt=kt: u_av(kt) for kt in range(max(nkt - LAG, 0), nkt)]
                tail += [u_norm_a, u_norm_b, u_norm_c]
                return units, tail

            def b_units(s):
                pending = pending_norms[:]
                units = []
                bounds = []
                for c in range(4):
                    if c > 0:
                        bounds.append(len(units))
                    tiles, tail = bc_units(s, c)
                    merged = []
                    for i, u in enumerate(tiles):
                        merged.append(u)
                        if i < len(pending):
                            merged.append(pending[i])
                    if len(pending) > len(tiles):
                        merged.extend(pending[len(tiles):])
                    units.extend(merged)
                    pending = tail
                pending_norms[:] = pending
                return units, bounds

            # ---- Phase C: partial projection (host reduces across head groups)
            ot_bigs = {}

            def c_units(s):
                def u_proj(o, n):
                    if s not in ot_bigs:
                        ot_bigs[s] = sb.tile(
                            [P, 8 * NQ], F16, tag="ot", bufs=2, name=f"ot{s}"
                        )
                    ps = psp.tile([P, NQ], f32, tag="mm", bufs=2, name="psc")
                    for c in range(4):
                        nc.tensor.matmul(
                            ps[:],
                            yts[(c, s)][:, o * P : (o + 1) * P],
                            wp_sb[c * 2 + n][:],
                            start=(c == 0),
                            stop=(c == 3),
                        )
                    k = o * 2 + n
                    nc.vector.tensor_copy(ot_bigs[s][:, k * NQ : (k + 1) * NQ], ps[:])
                    if k == 7:
                        nc.sync.dma_start(
                            out[:, s * 8 * NQ : (s + 1) * 8 * NQ], ot_bigs[s][:]
                        )

                return [lambda o=o, n=n: u_proj(o, n) for o in range(4) for n in range(2)]

            # ---- Driver: emit B(s) woven with C(s-1) then A(s+1) so the PE
            # queue alternates attention work with projection/qkv matmuls
            # (which fill PE while ACT runs the exps).
            def weave(primary, secondary, hold=2, bounds=()):
                # hold: emit this many primary units before the first
                # secondary one (the strip-crossing displaced norm units sit
                # in the first few primary slots and C units depend on them).
                # bounds: group-start indices -- force one secondary (PE
                # filler) right before each so the new group's first QK isn't
                # stalled on the exp that frees a qkp PSUM buffer. The
                # linear pacing is also front-loaded 1.25x so the next
                # strip's QT copies aren't emitted behind the last norm
                # chains on the DVE queue.
                np_, ns_ = len(primary), len(secondary)
                hold = min(hold, np_ - 1)
                emitted = 0
                for i, u in enumerate(primary):
                    if i in bounds and emitted < ns_:
                        secondary[emitted]()
                        emitted += 1
                    u()
                    want = max(0, (i + 1 - hold)) * ns_ // (np_ - hold)
                    want = min(ns_, (want * 5 + 3) // 4)
                    while emitted < want:
                        secondary[emitted]()
                        emitted += 1
                while emitted < ns_:
                    secondary[emitted]()
                    emitted += 1

            # C(s) is woven two strips later (C0->B2, C1/C2->B3): late strips
            # are locally ACT(exp)-bound, so they need the projection matmuls
            # as PE filler; early strips are PE-bound and don't.
            for u in a_units(0):
                u()
            for s in range(NSTRIP):
                others = []
                if s + 1 < NSTRIP:
                    others.extend(a_units(s + 1))
                if s == 2:
                    others.extend(c_units(0))
                elif s == 3:
                    others.extend(c_units(1))
                    others.extend(c_units(2))
                bu, bounds = b_units(s)
                weave(bu, others, bounds=set(bounds))
            for u in pending_norms:
                u()
            for u in c_units(NSTRIP - 1):
                u()

    nc.compile()
    return nc


_NC = None


def _get_nc():
    global _NC
    if _NC is None:
        _NC = build()
    return _NC


def host_mask():
    # cols 0-127: multiplicative causal mask for a diagonal 128x128 block
    # (keep k <= q); cols 128-255 unused
    m = np.zeros((P, 256), np.float16)
    for kk in range(P):
        m[kk, kk:128] = 1.0
    return m


def _pack_cblocks(a):
    """[8*128, n] -> [128, 8*n]: stack the 128-row blocks along columns."""
    n = a.shape[1]
    out = np.empty((P, 8 * n), a.dtype)
    for c in range(8):
        out[:, c * n : (c + 1) * n] = a[c * P : (c + 1) * P, :]
    return out


def make_in_maps(x, w_qkv, w_proj):
    x = np.asarray(x, np.float32)
    w_qkv = np.asarray(w_qkv, np.float16)
    w_proj = np.asarray(w_proj, np.float16)
    mkm = host_mask()
    in_maps = []
    for core in range(NCORES):
        b, hg = core // 2, core % 2
        lo, hi = hg * CH, (hg + 1) * CH
        xtb = x[b].T.astype(np.float16)  # (C, T)
        # xt packed per strip: [128, 4*8*512], strip s block = c-blocks of
        # xt[:, s*512:(s+1)*512] stacked along columns
        xtp = np.empty((P, NSTRIP * 8 * NQ), np.float16)
        for s in range(NSTRIP):
            xtp[:, s * 8 * NQ : (s + 1) * 8 * NQ] = _pack_cblocks(
                xtb[:, s * NQ : (s + 1) * NQ]
            )
        # wp packed: block k=c*2+n is w_proj[lo+c*128 : lo+(c+1)*128, n*512:(n+1)*512]
        wpc = w_proj[lo:hi, :]
        wpp = np.empty((P, 8 * NQ), np.float16)
        for c in range(4):
            for n in range(2):
                wpp[:, (c * 2 + n) * NQ : (c * 2 + n + 1) * NQ] = wpc[
                    c * P : (c + 1) * P, n * NQ : (n + 1) * NQ
                ]
        in_maps.append(
            {
                "xt": xtp,
                "wq": _pack_cblocks(w_qkv[:, lo:hi]),
                "wk": _pack_cblocks(w_qkv[:, C + lo : C + hi]),
                "wv": _pack_cblocks(w_qkv[:, 2 * C + lo : 2 * C + hi]),
                "wp": wpp,
                "mk": mkm,
            }
        )
    return in_maps


def kernel(x, w_qkv, w_proj):
    in_maps = make_in_maps(x, w_qkv, w_proj)
    last_err = None
    for attempt in range(3):
        try:
            res = bass_utils.run_bass_kernel_spmd(
                _get_nc(), in_maps, core_ids=list(range(NCORES))
            )
            break
        except Exception as e:  # transient device wedge: back off and retry
            last_err = e
            import time

            time.sleep(10 * (attempt + 1))
    else:
        raise last_err
    out = np.empty((B, T, C), np.float32)
    for b in range(B):
        pk = res.results[2 * b]["out"].astype(np.float32) + res.results[2 * b + 1][
            "out"
        ].astype(np.float32)
        # unpack [128, 32*512]: block j = (q-tile m)*2 + (c-col half n)
        for m in range(16):
            for n in range(2):
                j = m * 2 + n
                out[b, m * P : (m + 1) * P, n * NQ : (n + 1) * NQ] = pk[
                    :, j * NQ : (j + 1) * NQ
                ]
    return out



# revision 29
# speedup vs baseline: 1.0033x; 1.0033x over previous
"""Causal self-attention (B=4, T=2048, C=1024, 16 heads) on 8 TRN2 NeuronCores.

Sharding: data-parallel over batch (4) x tensor-parallel over heads (2 groups
of 8). Each core computes qkv + attention for its 8 heads and a partial
output projection (row-parallel); the host sums the two partials per batch.

Everything on-chip runs in a transposed layout so no tensor ever needs an
on-device transpose:
  QT/KT [ch, t]  <- W^T @ x^T      (x^T prepared on host)
  attT  [k, q]   = exp(K @ Q^T / 8) * causal_mask
  yT    [ch, q]  = V_aug^T @ attT  (V augmented with a ones column per head ->
                                    row 64 of each head's block = softmax denom)
  out   [q, c]   = yT^T @ Wp       (partial; host-reduced across head groups)

Softmax skips max-subtraction (scores/8 are O(1) here, exp is safe), which is
mathematically identical to the reference; fully-masked blocks are never
computed, straddle blocks only compute the causally valid column range, and
only the diagonal 128-wide sub-block needs a mask multiply (on GpSimd, which
is otherwise idle).

All matmul operands are fp16 (same 1 cyc/row PE rate as fp32r at N>=256 but
no 4x penalty at N=128, and half the DMA/SBUF traffic); PSUM accumulation and
the softmax normalization stay f32. Phase B is software-pipelined per k-tile:
QK+exp issue ahead, AV lags LAG tiles so the in-order PE queue never waits on
the Scalar-engine exp; the normalization units of each head-pair group are
displaced into the next group's tile stream for the same reason.
"""

import os
import sys

import numpy as np

for _p in ("/opt/trn_rl_repo", "/root/.axon_site/_ro/trn_rl_repo"):
    if os.path.isdir(_p) and _p not in sys.path:
        sys.path.append(_p)

import concourse.bass as bass  # noqa: E402,F401
import concourse.mybir as mybir  # noqa: E402
import concourse.tile as tile  # noqa: E402
from concourse import bacc, bass_utils  # noqa: E402

f32 = mybir.dt.float32
f32r = mybir.dt.float32r
F16 = mybir.dt.float16

B, T, C = 4, 2048, 1024
N_HEAD, D = 16, 64
NCORES = 8
HPC = 8  # heads per core
CH = HPC * D  # 512 channels per core
P = 128
NQ = 512  # q-strip width
NSTRIP = T // NQ  # 4
SCALE = 1.0 / 8.0  # 1/sqrt(D)
LAG = 5  # AV trails QK/exp by this many k-tiles in the PE stream

MM_DT = F16

# exp(u/8) ~= p(u)^8 with p = rel-minimax quadratic of exp(u/64) on
# [-21.5, 21.5] (measured max |raw score| is 20.37). Runs on the DVE as a
# custom op (8-stage budget: quad + 3 squarings) so the Scalar engine's exp
# stream stops being the critical path in the late (long-context) strips.
# Max rel err 1.3e-2 on ~1/4 of the attention weights -> ~2e-3 on the output.
EK0 = 1.000390887260437
EK1 = 0.01584288291633129
EK2 = 0.0001212068455060944


def register_exp_op():
    """Register the EXP8_ANT custom DVE op (idempotent)."""
    from concourse import dve_ops as D
    from concourse.dve_spec import Spec, Src0, sq, C0, C1, C2, _has_src1, lower
    from concourse.dve_uop import DveOpSpec

    for op in D.OPS:
        if op.name == "EXP8_ANT":
            return op

    _p = (C0 + Src0 * C1) + sq(Src0) * C2
    _body = sq(sq(sq(_p)))

    def _ref(in0, in1, s0, s1, imm2):
        p = (s0 + in0 * s1) + in0 * in0 * imm2
        return ((p * p) ** 2) ** 2

    spec = Spec(body=_body, reference=_ref)
    op = D.DveOp("EXP8_ANT", spec, subdim=False, uops_sha={})
    D.OPS.append(op)
    # import-time derived registries must be refreshed for the new row
    D._SUB_OPCODE_FOR_NAME[op.name] = D._CUSTOM_DVE_ROW_BASE + len(D.OPS) - 1
    D.CUSTOM_DVE_SPECS[op.name] = spec
    # pre-seed the compile cache so the empty sha dict is never checked
    for ver in ("v3", "v4"):
        try:
            uops = lower(spec, ver=ver)
        except Exception:
            continue
        D._COMPILE_CACHE[(op.name, ver)] = DveOpSpec(
            name=op.name,
            opcode=D.get_dve_sub_opcode(op.name),
            uops=uops,
            rd1_en=_has_src1(spec),
        )
    return op


def build():
    nc = bacc.Bacc("TRN2", target_bir_lowering=False, debug=False)
    # All operands arrive pre-packed host-side as [128, n*512] with the
    # 128-partition c-blocks stacked along columns: every DMA then moves
    # 8KB/partition of contiguous dram per tile instead of 1KB rows, which
    # cuts descriptor count 8x and gets the first QT chain its inputs
    # ~6us earlier.
    xt = nc.dram_tensor("xt", (P, NSTRIP * 8 * NQ), MM_DT, kind="ExternalInput")
    wq = nc.dram_tensor("wq", (P, 8 * CH), MM_DT, kind="ExternalInput")
    wk = nc.dram_tensor("wk", (P, 8 * CH), MM_DT, kind="ExternalInput")
    wv = nc.dram_tensor("wv", (P, 8 * CH), MM_DT, kind="ExternalInput")
    wp = nc.dram_tensor("wp", (P, 8 * NQ), MM_DT, kind="ExternalInput")
    # mk: cols 0-127 identity, cols 128-255 additive causal mask
    # (0 keep / -60000 drop)
    mk = nc.dram_tensor("mk", (P, 256), F16, kind="ExternalInput")
    # out packed the same way: strip s holds blocks k=o*2+n (o = q-tile
    # within strip, n = c-col half) at cols (s*8+k)*NQ
    out = nc.dram_tensor("out", (P, 32 * NQ), F16, kind="ExternalOutput")
    Exp = mybir.ActivationFunctionType.Exp
    exp_op = register_exp_op()

    with tile.TileContext(nc) as tc:
        with (
            tc.tile_pool(name="sb", bufs=1) as sb,
            tc.tile_pool(name="ps", bufs=1, space="PSUM") as psp,
        ):
            mask = sb.tile([P, 256], F16, tag="mask", bufs=1, name="mask")
            nc.sync.dma_start(mask[:], mk[:])
            maskmul = mask[:, 0:128]
            # PE warmup: stream zeros through the PE while the weight/x DMAs
            # land so the tensor engine p-state ramps before real work.
            # memset on GpSimd (its preamble finishes earliest and nothing
            # else queues there) so the first warmup matmul isn't gated on
            # the DVE preamble + memsets. ~14 cold matmuls (~6us) cover the
            # HAM busy-window flip; the DMAs land at about the same time.
            wrm = sb.tile([P, NQ], F16, tag="wrm", bufs=1, name="wrm")
            nc.gpsimd.memset(wrm[:], 0.0)
            # 34 warmups bridge the PE from its preamble (~7.4us) to the
            # first-load DMA landing (~17.5us): an idle hole here re-throttles
            # HAM and the first real chains then run at half clock.
            for i in range(34):
                wps_ = psp.tile([P, NQ], f32, tag="mm", bufs=2, name="wrmp")
                nc.tensor.matmul(wps_[:], wrm[:, 0:P], wrm[:], start=True, stop=True)
            sel = sb.tile([33, P], F16, tag="sel", bufs=1, name="sel")
            nc.vector.memset(sel[:], 0.0)
            nc.vector.memset(sel[0:1, 0:64], 1.0)
            nc.vector.memset(sel[32:33, 64:128], 1.0)
            col1 = sb.tile([P, HPC], F16, tag="ones8", bufs=1, name="col1")
            nc.vector.memset(col1[:], 1.0)
            # den rows 1..31 must be a harmless non-zero constant for the
            # reciprocal (sel zeroes them in the broadcast); memset once,
            # rows 0/32 are overwritten per group.
            dens = [
                sb.tile([33, NQ], f32, tag="den", bufs=2, name=f"den{i}")
                for i in range(2)
            ]
            for dt_ in dens:
                nc.vector.memset(dt_[:], 1.0)

            # ---- weight + first-strip x loads, interleaved so the first QT
            # accumulation chain unblocks after two small DMAs. x issues on
            # the (startup-idle) Activation engine in parallel with Sync.
            wq_sb, wk_sb, wv_sb = ([] for _ in range(3))
            xts = {}  # strip -> list of 8 c-tiles
            xdma_done = set()

            def x_dma(s, eng):
                if s in xdma_done or s >= NSTRIP:
                    return
                xdma_done.add(s)
                t = sb.tile([P, 8 * NQ], MM_DT, tag="x", bufs=3, name=f"x{s}")
                eng.dma_start(t[:], xt[:, s * 8 * NQ : (s + 1) * 8 * NQ])
                xts[s] = [t[:, c * NQ : (c + 1) * NQ] for c in range(8)]

            wqbig = sb.tile([P, 8 * CH], MM_DT, tag="w", bufs=4, name="wq")
            nc.sync.dma_start(wqbig[:], wq[:])
            wq_sb = [wqbig[:, c * CH : (c + 1) * CH] for c in range(8)]
            x_dma(0, nc.scalar)
            wkbig = sb.tile([P, 8 * CH], MM_DT, tag="w", bufs=4, name="wk")
            nc.sync.dma_start(wkbig[:], wk[:])
            wk_sb = [wkbig[:, c * CH : (c + 1) * CH] for c in range(8)]
            wvbig = sb.tile([P, 8 * CH], MM_DT, tag="w", bufs=4, name="wv")
            nc.sync.dma_start(wvbig[:], wv[:])
            wv_sb = [wvbig[:, c * CH : (c + 1) * CH] for c in range(8)]
            wpbig = sb.tile([P, 8 * NQ], MM_DT, tag="w", bufs=4, name="wp")
            nc.scalar.dma_start(wpbig[:], wp[:])
            wp_sb = [wpbig[:, k * NQ : (k + 1) * NQ] for k in range(8)]

            kts = [sb.tile([P, T], MM_DT, tag="kt", bufs=4, name=f"kt{m}") for m in range(4)]
            vts = [None] * 16
            qts = {}

            # ---- Phase A: QT (strip-local), KT (transposed) and V (ones-augmented)
            def a_units(s):
                def u_dma():
                    x_dma(s, nc.sync)
                    x_dma(s + 1, nc.sync)
                    qts[s] = []

                def u_qt(m):
                    ps = psp.tile([P, NQ], f32, tag="mm", bufs=2, name="psa")
                    for c in range(8):
                        nc.tensor.matmul(
                            ps[:],
                            wq_sb[c][:, m * P : (m + 1) * P],
                            xts[s][c][:],
                            start=(c == 0),
                            stop=(c == 7),
                        )
                    qt_t = sb.tile([P, NQ], MM_DT, tag="qt", bufs=8, name=f"q{s}_{m}")
                    nc.vector.tensor_copy(qt_t[:], ps[:])
                    qts[s].append(qt_t)

                def u_kt(m):
                    ps = psp.tile([P, NQ], f32, tag="mm", bufs=2, name="psk")
                    for c in range(8):
                        nc.tensor.matmul(
                            ps[:],
                            wk_sb[c][:, m * P : (m + 1) * P],
                            xts[s][c][:],
                            start=(c == 0),
                            stop=(c == 7),
                        )
                    nc.vector.tensor_copy(kts[m][:, s * NQ : (s + 1) * NQ], ps[:])

                def u_v(mt):
                    g = s * 4 + mt
                    ps = psp.tile([P, NQ], f32, tag="mm", bufs=2, name="psv")
                    for c in range(8):
                        nc.tensor.matmul(
                            ps[:],
                            xts[s][c][:, mt * P : (mt + 1) * P],
                            wv_sb[c][:],
                            start=(c == 0),
                            stop=(c == 7),
                        )
                    vt = sb.tile([P, HPC * 65], F16, tag="v", bufs=16, name=f"v{g}")
                    v3 = vt.rearrange("p (h e) -> p h e", e=65)
                    # V copy must stay off ACT: queued ahead of ready exps it
                    # head-of-line blocks the attention pipeline.
                    nc.vector.tensor_copy(v3[:, :, 0:64], ps.rearrange("p (h e) -> p h e", e=64))
                    nc.vector.tensor_copy(
                        v3[:, :, 64:65], col1[:].rearrange("p (h e) -> p h e", e=1)
                    )
                    vts[g] = vt

                units = [u_dma]
                for m in range(4):
                    units.append(lambda m=m: u_qt(m))
                    units.append(lambda m=m: u_kt(m))
                    units.append(lambda m=m: u_v(m))
                return units

            # ---- Phase B: flash attention, software-pipelined per k-tile.
            yts = {}
            pending_norms = []  # norm units of the previous head-pair group

            def bc_units(s, c):
                nkt = 4 * (s + 1)
                st = {}

                def u_tile(kt):
                    j = kt - 4 * s  # >=0: diagonal-straddling tile
                    off = 128 * max(j, 0)
                    qkp = psp.tile([P, 2 * NQ], f32, tag="qk", bufs=2, name="qkp")
                    att = sb.tile([P, 2 * NQ], F16, tag="att", bufs=LAG + 3, name="att")
                    for sub in range(2):
                        nc.tensor.matmul(
                            qkp[:, sub * NQ + off : (sub + 1) * NQ],
                            kts[c][sub * 64 : (sub + 1) * 64, kt * P : (kt + 1) * P],
                            qts[s][c][sub * 64 : (sub + 1) * 64, off:NQ],
                            start=True,
                            stop=True,
                        )
                    # late strips are Scalar-exp-bound: route ~1/4 of the
                    # k-tiles' exps through the DVE poly op instead.
                    dve_exp = False
                    if dve_exp:
                        nc.vector._custom_dve(
                            exp_op,
                            out=att.rearrange("p (u q) -> p u q", u=2)[:, :, off:NQ],
                            in0=qkp.rearrange("p (u q) -> p u q", u=2)[:, :, off:NQ],
                            s0=EK0,
                            s1=EK1,
                            imm2=EK2,
                        )
                    else:
                        nc.scalar.activation(
                            att.rearrange("p (u q) -> p u q", u=2)[:, :, off:NQ],
                            qkp.rearrange("p (u q) -> p u q", u=2)[:, :, off:NQ],
                            Exp,
                            scale=SCALE,
                        )
                    if j >= 0:
                        for sub in range(2):
                            nc.gpsimd.tensor_mul(
                                att[:, sub * NQ + off : sub * NQ + off + 128],
                                att[:, sub * NQ + off : sub * NQ + off + 128],
                                maskmul,
                            )
                    st[kt] = (att, off)

                def u_av(kt):
                    if "av" not in st:
                        st["av"] = [
                            psp.tile([65, NQ], f32, tag="sm", bufs=2, name=f"av{s}{c}{u}")
                            for u in range(2)
                        ]
                    att, off = st.pop(kt)
                    for sub in range(2):
                        h = 2 * c + sub
                        nc.tensor.matmul(
                            st["av"][sub][:, off:NQ],
                            vts[kt][:, h * 65 : (h + 1) * 65],
                            att[:, sub * NQ + off : (sub + 1) * NQ],
                            start=(kt == 0),
                            stop=(kt == nkt - 1),
                        )

                def u_norm_a():
                    # denominators -> reciprocals (DVE), cast f16 for the
                    # 1 cyc/row selector matmul
                    av = st["av"]
                    den = dens[(4 * s + c) % 2]
                    rcf = sb.tile([33, NQ], f32, tag="rcf", bufs=2, name="rcf")
                    rc2 = sb.tile([33, NQ], F16, tag="rc2", bufs=2, name="rc2")
                    for sub in range(2):
                        nc.vector.tensor_copy(
                            den[32 * sub : 32 * sub + 1, :], av[sub][64:65, :]
                        )
                    nc.vector.reciprocal_approx_fast(out=rcf[:], in_=den[:])
                    nc.vector.tensor_copy(rc2[:], rcf[:])
                    st["rc2"] = rc2

                def u_norm_b():
                    # broadcast reciprocals to 128 partitions via selector matmul
                    bc_ps = psp.tile([P, NQ], f32, tag="mm", bufs=2, name="bcp")
                    nc.tensor.matmul(bc_ps[:], sel[:], st["rc2"][:], start=True, stop=True)
                    st["bc_ps"] = bc_ps

                def u_norm_c():
                    bc = sb.tile([P, NQ], f32, tag="bc", bufs=2, name="bc")
                    nc.vector.tensor_copy(bc[:], st["bc_ps"][:])
                    av = st["av"]
                    yts[(c, s)] = sb.tile([P, NQ], MM_DT, tag="yt", bufs=16, name=f"y{c}{s}")
                    for sub in range(2):
                        nc.vector.tensor_mul(
                            yts[(c, s)][sub * 64 : (sub + 1) * 64, :],
                            av[sub][0:64, :],
                            bc[sub * 64 : (sub + 1) * 64, :],
                        )

                units = []
                for kt in range(nkt):
                    def u(kt=kt):
                        u_tile(kt)
                        if kt - LAG >= 0:
                            u_av(kt - LAG)
                    units.append(u)
                # AV tail (exp-gated) + norm chain are displaced into the
                # next group's tile stream: its early QKs are independent of
                # this group's last exps, so the PE never idles on them.
                # Every tail unit carries a PE instruction (norm_a rides with
                # the last AV, norm_c with the bcast): a PE-less unit between
                # two QK pairs makes the pairs adjacent in the PE queue, and
                # back-to-back 64-row-mode pairs stall ~450ns on the drain.
                tail = [lambda kt=kt: u_av(kt) for kt in range(max(nkt - LAG, 0), nkt)]
                tail += [u_norm_a, u_norm_b, u_norm_c]
                return units, tail

            def b_units(s):
                pending = pending_norms[:]
                units = []
                for c in range(4):
                    tiles, tail = bc_units(s, c)
                    merged = []
                    for i, u in enumerate(tiles):
                        merged.append(u)
                        if i < len(pending):
                            merged.append(pending[i])
                    if len(pending) > len(tiles):
                        merged.extend(pending[len(tiles):])
                    units.extend(merged)
                    pending = tail
                pending_norms[:] = pending
                return units

            # ---- Phase C: partial projection (host reduces across head groups)
            ot_bigs = {}

            def c_units(s):
                def u_proj(o, n):
                    if s not in ot_bigs:
                        ot_bigs[s] = sb.tile(
                            [P, 8 * NQ], F16, tag="ot", bufs=2, name=f"ot{s}"
                        )
                    ps = psp.tile([P, NQ], f32, tag="mm", bufs=2, name="psc")
                    for c in range(4):
                        nc.tensor.matmul(
                            ps[:],
                            yts[(c, s)][:, o * P : (o + 1) * P],
                            wp_sb[c * 2 + n][:],
                            start=(c == 0),
                            stop=(c == 3),
                        )
                    k = o * 2 + n
                    nc.vector.tensor_copy(ot_bigs[s][:, k * NQ : (k + 1) * NQ], ps[:])
                    if k == 7:
                        nc.sync.dma_start(
                            out[:, s * 8 * NQ : (s + 1) * 8 * NQ], ot_bigs[s][:]
                        )

                return [lambda o=o, n=n: u_proj(o, n) for o in range(4) for n in range(2)]

            # ---- Driver: emit B(s) woven with C(s-1) then A(s+1) so the PE
            # queue alternates attention work with projection/qkv matmuls
            # (which fill PE while ACT runs the exps).
            def weave(primary, secondary, hold=2):
                # hold: emit this many primary units before the first
                # secondary one (the strip-crossing displaced norm units sit
                # in the first few primary slots and C units depend on them)
                np_, ns_ = len(primary), len(secondary)
                hold = min(hold, np_ - 1)
                emitted = 0
                for i, u in enumerate(primary):
                    u()
                    want = max(0, (i + 1 - hold)) * ns_ // (np_ - hold)
                    while emitted < want:
                        secondary[emitted]()
                        emitted += 1
                while emitted < ns_:
                    secondary[emitted]()
                    emitted += 1

            # C(s) is woven two strips later (C0->B2, C1/C2->B3): late strips
            # are locally ACT(exp)-bound, so they need the projection matmuls
            # as PE filler; early strips are PE-bound and don't.
            for u in a_units(0):
                u()
            for s in range(NSTRIP):
                others = []
                if s + 1 < NSTRIP:
                    others.extend(a_units(s + 1))
                if s == 2:
                    others.extend(c_units(0))
                elif s == 3:
                    others.extend(c_units(1))
                    others.extend(c_units(2))
                weave(b_units(s), others)
            for u in pending_norms:
                u()
            for u in c_units(NSTRIP - 1):
                u()

    nc.compile()
    return nc


_NC = None


def _get_nc():
    global _NC
    if _NC is None:
        _NC = build()
    return _NC


def host_mask():
    # cols 0-127: multiplicative causal mask for a diagonal 128x128 block
    # (keep k <= q); cols 128-255 unused
    m = np.zeros((P, 256), np.float16)
    for kk in range(P):
        m[kk, kk:128] = 1.0
    return m


def _pack_cblocks(a):
    """[8*128, n] -> [128, 8*n]: stack the 128-row blocks along columns."""
    n = a.shape[1]
    out = np.empty((P, 8 * n), a.dtype)
    for c in range(8):
        out[:, c * n : (c + 1) * n] = a[c * P : (c + 1) * P, :]
    return out


def make_in_maps(x, w_qkv, w_proj):
    x = np.asarray(x, np.float32)
    w_qkv = np.asarray(w_qkv, np.float16)
    w_proj = np.asarray(w_proj, np.float16)
    mkm = host_mask()
    in_maps = []
    for core in range(NCORES):
        b, hg = core // 2, core % 2
        lo, hi = hg * CH, (hg + 1) * CH
        xtb = x[b].T.astype(np.float16)  # (C, T)
        # xt packed per strip: [128, 4*8*512], strip s block = c-blocks of
        # xt[:, s*512:(s+1)*512] stacked along columns
        xtp = np.empty((P, NSTRIP * 8 * NQ), np.float16)
        for s in range(NSTRIP):
            xtp[:, s * 8 * NQ : (s + 1) * 8 * NQ] = _pack_cblocks(
                xtb[:, s * NQ : (s + 1) * NQ]
            )
        # wp packed: block k=c*2+n is w_proj[lo+c*128 : lo+(c+1)*128, n*512:(n+1)*512]
        wpc = w_proj[lo:hi, :]
        wpp = np.empty((P, 8 * NQ), np.float16)
        for c in range(4):
            for n in range(2):
                wpp[:, (c * 2 + n) * NQ : (c * 2 + n + 1) * NQ] = wpc[
                    c * P : (c + 1) * P, n * NQ : (n + 1) * NQ
                ]
        in_maps.append(
            {
                "xt": xtp,
                "wq": _pack_cblocks(w_qkv[:, lo:hi]),
                "wk": _pack_cblocks(w_qkv[:, C + lo : C + hi]),
                "wv": _pack_cblocks(w_qkv[:, 2 * C + lo : 2 * C + hi]),
                "wp": wpp,
                "mk": mkm,
            }
        )
    return in_maps


def kernel(x, w_qkv, w_proj):
    in_maps = make_in_maps(x, w_qkv, w_proj)
    last_err = None
    for attempt in range(3):
        try:
            res = bass_utils.run_bass_kernel_spmd(
                _get_nc(), in_maps, core_ids=list(range(NCORES))
            )
            break
        except Exception as e:  # transient device wedge: back off and retry
            last_err = e
            import time

            time.sleep(10 * (attempt + 1))
    else:
        raise last_err
    out = np.empty((B, T, C), np.float32)
    for b in range(B):
        pk = res.results[2 * b]["out"].astype(np.float32) + res.results[2 * b + 1][
            "out"
        ].astype(np.float32)
        # unpack [128, 32*512]: block j = (q-tile m)*2 + (c-col half n)
        for m in range(16):
            for n in range(2):
                j = m * 2 + n
                out[b, m * P : (m + 1) * P, n * NQ : (n + 1) * NQ] = pk[
                    :, j * NQ : (j + 1) * NQ
                ]
    return out

